# revision 53
# baseline (speedup 1.0000x reference)
"""GQA sparse-attention (sink + sliding window) kernel for 8 TRN2 NeuronCores.

Problem: nn_MultiHeadSelfAttentionModern (B=1, T=2048, D=2048, 32 q heads,
8 KV heads, d_head=64, WINDOW=2048, SINK=64, start_pos=2048, cache_len=2048).

Since S = cache_len + T = 4096 > WINDOW + SINK = 2112, the effective keys are
just kv_cache[:, :, :64] (the sink, used raw for both K and V) plus the 2048
new RoPE'd k (and raw new v).  Sharding: tensor-parallel by KV head - core i
owns KV head i and its 4 query heads, with Wq/Wk/Wv column-sharded and Wo
row-sharded; partial (bf16) outputs are summed on the host (+ bo).

Cost-model-optimized dataflow, software-pipelined in rounds:
  - projections in bf16 (x, Wq, Wkv host-cast) with k|v merged into one
    128-column psum stream; RoPE per 512-token chunk on DVE.
  - per (head, 1024-token half) block B: scores_T[s,t] on PE -> exp on ACT
    (the ~145us exp stream is the pacer); ctx is computed TRANSPOSED:
    ctx[t, d+1] += wT_st.T @ (v_st|ones) charges only 65 PE rows per matmul,
    accumulated tile-major (one open psum accumulation group per bank).
    The ones column yields softmax denominators per token-partition.
  - normalize = reciprocal + tensor_scalar_mul, PE-transpose back to [d, t]
    (odd heads hop to partitions 64-127 via one SBUF-SBUF DMA per block),
    y = ctxT.T @ Wo in psum, staged to SBUF (DVE, ACT at the tail) and
    DMA'd out as bf16.
"""

import numpy as np

T = 2048
DMODEL = 2048
NKV = 8
GROUP = 4
DH = 64
SINK = 64
NST = 17  # s-tiles: 16 full 128-tiles of new tokens + 1 sink tile (64 rows)
SCALE = 0.125  # 1/sqrt(64)

# ctx psum slot offsets (even-aligned: HW requires even psum element bases).
# 7 slots in bank A + 1 at the base of bank B; transpose staging at 640/768.
CTX_SLOTS = [0, 66, 132, 198, 264, 330, 396, 512]
TPS_SLOTS = [640, 768]
# ctx consumes s-tiles in the order their exps complete: sink first
ST_ORDER = [NST - 1] + list(range(16))

_CACHE = {}


def _interleave(*gens):
    """Round-robin the generators: one quantum each per cycle.

    Each entry is either a generator or (generator, start_delay_cycles).
    """
    slots = []
    for g in gens:
        if g is None:
            continue
        if isinstance(g, tuple):
            slots.append([g[0], g[1]])
        else:
            slots.append([g, 0])
    cycle = 0
    while slots:
        keep = []
        for ent in slots:
            g, delay = ent
            if cycle < delay:
                keep.append(ent)
                continue
            try:
                next(g)
                keep.append(ent)
            except StopIteration:
                pass
        slots = keep
        cycle += 1


def _build_nc():
    import concourse.bass as bass
    import concourse.mybir as mybir
    import concourse.tile as tile
    from concourse import bacc
    from concourse.masks import make_identity

    f32 = mybir.dt.float32
    f32r = mybir.dt.float32r
    bf16 = mybir.dt.bfloat16

    nc = bacc.Bacc("TRN2", target_bir_lowering=False, debug=False, num_devices=NKV)

    xT = nc.declare_dram_parameter("xT", [DMODEL, T], bf16, isOutput=False).ap()
    wq = nc.declare_dram_parameter("wq", [DMODEL, GROUP * DH], bf16, isOutput=False).ap()
    wkv = nc.declare_dram_parameter("wkv", [DMODEL, 2 * DH], bf16, isOutput=False).ap()
    wo = nc.declare_dram_parameter("wo", [GROUP * DH, DMODEL], f32, isOutput=False).ap()
    sink_kT = nc.declare_dram_parameter("sink_kT", [DH, SINK], f32r, isOutput=False).ap()
    sink_v = nc.declare_dram_parameter("sink_v", [SINK, DH], f32, isOutput=False).ap()
    cosb = nc.declare_dram_parameter("cosb", [128, T], bf16, isOutput=False).ap()
    sinb = nc.declare_dram_parameter("sinb", [128, T], bf16, isOutput=False).ap()
    out = nc.declare_dram_parameter("out", [T, DMODEL], bf16, isOutput=True).ap()

    # processing order of the 8 (head, half) blocks: half-major, odd
    # heads first so the final norms (gating yout) need no partition hop
    BLOCKS = [(1, 0), (3, 0), (0, 0), (2, 0), (1, 1), (3, 1), (0, 1), (2, 1)]

    with (
        tile.TileContext(nc) as tc,
        tc.tile_pool(name="persist", bufs=1) as persist,
        tc.tile_pool(name="psS", bufs=2, space="PSUM") as psS,
        tc.tile_pool(name="pm", bufs=1) as pm,
    ):
        q01 = persist.tile([128, T], f32r, tag="q01")
        q23 = persist.tile([128, T], f32r, tag="q23")
        qx1 = persist.tile([DH, T], f32r, tag="qx1")
        qx3 = persist.tile([DH, T], f32r, tag="qx3")
        kk = persist.tile([DH, T], f32r, tag="kk")
        v_sb = persist.tile([128, NST, DH + 1], bf16, tag="v_sb")
        ctxT = [persist.tile([128, T], bf16, tag=f"ctxT{j}", name=f"ctxT{j}") for j in range(2)]
        ident64 = persist.tile([SINK, SINK], f32, tag="ident64")
        identB = persist.tile([128, 128], f32, tag="identB")
        sink_kT_sb = persist.tile([DH, SINK], f32r, tag="sink_kT")
        cos_sb = persist.tile([128, T], bf16, tag="cos_sb")
        sin_sb = persist.tile([128, T], bf16, tag="sin_sb")
        recip_sb = persist.tile([128, 64], f32, tag="recip_sb")
        zero_sb = persist.tile([128, 1], f32, tag="zero_sb")
        nc.vector.memset(zero_sb, 0.0)

        xTr = xT.rearrange("(k p) t -> p k t", p=128)
        wkvr = wkv.rearrange("(k p) m -> p k m", p=128)
        wqr = wq.rearrange("(k p) m -> p k m", p=128)

        qsrc = [q01[0:DH, :], qx1, q23[0:DH, :], qx3]
        wT_tiles = {}
        cps_tiles = {}

        def get_wT(bi):
            if bi not in wT_tiles:
                h, half = BLOCKS[bi]
                wT_tiles[bi] = pm.tile(
                    [128, NST, 1024], bf16, tag="wT", bufs=3, name=f"wT{h}_{half}"
                )
            return wT_tiles[bi]

        def scores_gen(bi, sts):
            """One quantum per item: item = st or (st, u-half)."""
            h, half = BLOCKS[bi]
            wT = get_wT(bi)
            qt = qsrc[h]
            c0 = half * 1024
            for item in sts:
                st, uh = item if isinstance(item, tuple) else (item, None)
                p = SINK if st == NST - 1 else 128
                sps = psS.tile([128, 1024], f32, tag="sps", name="sps")
                lhsT = sink_kT_sb if st == NST - 1 else kk[:, st * 128 : (st + 1) * 128]
                us = range(2) if uh is None else (uh,)
                for u in us:
                    nc.tensor.matmul(
                        sps[0:p, u * 512 : (u + 1) * 512],
                        lhsT=lhsT,
                        rhs=qt[:, c0 + u * 512 : c0 + (u + 1) * 512],
                        start=True,
                        stop=True,
                    )
                if uh is None:
                    nc.scalar.activation(
                        out=wT[0:p, st, :],
                        in_=sps[0:p, :],
                        func=mybir.ActivationFunctionType.Exp,
                        bias=zero_sb[0:p, :],
                        scale=SCALE,
                    )
                else:
                    nc.scalar.activation(
                        out=wT[0:p, st, uh * 512 : (uh + 1) * 512],
                        in_=sps[0:p, uh * 512 : (uh + 1) * 512],
                        func=mybir.ActivationFunctionType.Exp,
                        bias=zero_sb[0:p, :],
                        scale=SCALE,
                    )
                yield

        def ctx_gen(bi):
            """tl-major ctx accumulation: 17 matmuls per token-tile slot
            (one open psum accumulation group per bank region at a time)."""
            h, half = BLOCKS[bi]
            wT = get_wT(bi)
            cps = psC.tile([128, 1024], f32, tag="cps", name=f"cps{h}_{half}")
            cps_tiles[bi] = (cps, [(cps, CTX_SLOTS[tl]) for tl in range(8)])
            for tl in range(8):
                for si, st in enumerate(ST_ORDER):
                    p = SINK if st == NST - 1 else 128
                    nc.tensor.matmul(
                        cps[:, CTX_SLOTS[tl] : CTX_SLOTS[tl] + DH + 1],
                        lhsT=wT[0:p, st, tl * 128 : (tl + 1) * 128],
                        rhs=v_sb[0:p, st, :],
                        start=(si == 0),
                        stop=(si == NST - 1),
                    )
                yield

        def norm_gen(bi, act_tail=False, per_slot=False):
            """Normalize + transpose ctx back to [d, t] bf16 in ctxT."""
            h, half = BLOCKS[bi]
            cps, slots = cps_tiles.pop(bi)
            roff = h * 16 + half * 8
            if per_slot:
                # per-slot recips: each tile's normalize becomes ready right
                # after its own ctx slot instead of after all eight
                for tl, (tile_, off) in enumerate(slots):
                    nc.vector.reciprocal(
                        recip_sb[:, roff + tl : roff + tl + 1],
                        tile_[:, off + DH : off + DH + 1],
                    )
            else:
                row = cps[:, DH : DH + 1]
                sums_a = bass.AP(
                    tensor=row.tensor, offset=row.offset,
                    ap=[row.ap[0]] + [[66, 7]],
                )
                nc.vector.reciprocal(recip_sb[:, roff : roff + 7], sums_a)
                nc.vector.reciprocal(
                    recip_sb[:, roff + 7 : roff + 8],
                    cps[:, CTX_SLOTS[7] + DH : CTX_SLOTS[7] + DH + 1],
                )
            yield
            codd = None
            if h % 2 == 1:
                codd = pm.tile([DH, 1024], bf16, tag="codd", bufs=2, name="codd")
            for tl in range(8):
                stile, slot = slots[tl]
                tt = half * 8 + tl
                ctxn = pm.tile([128, DH], f32, tag="ctxn", bufs=4, name="ctxn")
                nc.vector.tensor_scalar_mul(
                    out=ctxn,
                    in0=stile[:, slot : slot + DH],
                    scalar1=recip_sb[:, roff + tl : roff + tl + 1],
                )
                tslot = TPS_SLOTS[tl % 2]
                nc.tensor.transpose(cps[0:DH, tslot : tslot + 128], ctxn, identB)
                if h % 2 == 0:
                    nc.vector.tensor_copy(
                        out=ctxT[h // 2][0:DH, tt * 128 : (tt + 1) * 128],
                        in_=cps[0:DH, tslot : tslot + 128],
                    )
                elif act_tail:
                    nc.scalar.copy(
                        out=codd[:, tl * 128 : (tl + 1) * 128],
                        in_=cps[0:DH, tslot : tslot + 128],
                    )
                else:
                    nc.vector.tensor_copy(
                        out=codd[:, tl * 128 : (tl + 1) * 128],
                        in_=cps[0:DH, tslot : tslot + 128],
                    )
                if tl % 2 == 1:
                    if h % 2 == 1:
                        # lane-aligned engines can't shift partitions; DMA
                        # hops each finished pair to partitions 64-127
                        nc.gpsimd.dma_start(
                            out=ctxT[h // 2][
                                DH:128,
                                half * 1024 + (tl - 1) * 128
                                : half * 1024 + (tl + 1) * 128,
                            ],
                            in_=codd[:, (tl - 1) * 128 : (tl + 1) * 128],
                        )
                    yield
            yield

        def yout_gen(half, tls, act_tail=False):
            for tl in tls:
                tt = half * 8 + tl
                y_sb = pLate.tile([128, DMODEL], bf16, tag="y_sb", bufs=2,
                                  name="y_sb")
                for nck in range(4):
                    yps = psY.tile([128, 512], f32, tag="yps", name="yps")
                    for j in range(2):
                        nc.tensor.matmul(
                            yps,
                            lhsT=ctxT[j][:, tt * 128 : (tt + 1) * 128],
                            rhs=wo_sb[:, j, nck * 512 : (nck + 1) * 512],
                            start=(j == 0),
                            stop=(j == 1),
                        )
                    dst = y_sb[:, nck * 512 : (nck + 1) * 512]
                    if act_tail and nck % 2 == 1:
                        # ACT's exp stream is over; share the tail copies
                        nc.scalar.copy(out=dst, in_=yps)
                    else:
                        nc.vector.tensor_copy(out=dst, in_=yps)
                    if nck % 2 == 1:
                        yield
                nc.sync.dma_start(
                    out=out[tt * 128 : (tt + 1) * 128, :], in_=y_sb
                )

        # ---- phase 1: projections + per-chunk rope + v transpose ----
        with (
            tc.tile_pool(name="psB", bufs=1, space="PSUM") as psB,
            tc.tile_pool(name="pw", bufs=1) as pw,
            tc.tile_pool(name="px", bufs=2) as px,
        ):
            wq_sb = pw.tile([128, 16, GROUP * DH], bf16, tag="wq_sb")
            wkv_sb = pw.tile([128, 16, 2 * DH], bf16, tag="wkv_sb")
            # v^T staged on partitions 64-127 (straight DVE copy from the kv
            # psum stream); idv holds a 64x64 identity on partitions 64-127
            vT64 = pw.tile([128, T], f32, tag="vT64")
            idv = pw.tile([128, DH], f32, tag="idv")
            sinkv_st = pw.tile([SINK, DH], f32, tag="sinkv_st")
            warm = px.tile([128, 512], bf16, tag="warm")
            nc.vector.memset(warm, 0.25)
            nc.sync.dma_start(out=wkv_sb[:, 0:8], in_=wkvr[:, 0:8])

            def rope_chunk(tgt, cs, pp, eng=None):
                """tgt[0:pp, cs] <- tgt*C + swap32(tgt)*S on token slice cs."""
                eng = eng or nc.gpsimd
                sw = px.tile([128, 512], f32r, tag="sw", bufs=2)
                n = cs.stop - cs.start
                for b in range(pp // 64):
                    eng.dma_start(
                        out=sw[b * 64 : b * 64 + 32, 0:n],
                        in_=tgt[b * 64 + 32 : b * 64 + 64, cs],
                    )
                    eng.dma_start(
                        out=sw[b * 64 + 32 : b * 64 + 64, 0:n],
                        in_=tgt[b * 64 : b * 64 + 32, cs],
                    )
                nc.vector.tensor_mul(sw[0:pp, 0:n], sw[0:pp, 0:n], sin_sb[0:pp, cs])
                nc.vector.tensor_mul(tgt[0:pp, cs], tgt[0:pp, cs], cos_sb[0:pp, cs])
                nc.vector.tensor_add(tgt[0:pp, cs], tgt[0:pp, cs], sw[0:pp, 0:n])

            xt_tiles = {}

            def proj_dma(c):
                """Queue the 8 x-piece loads of chunk c (cross-chunk prefetch)."""
                xts = []
                for piece in range(8):
                    xt = px.tile([128, 2, 512], bf16, tag="xt", bufs=7)
                    nc.sync.dma_start(
                        out=xt,
                        in_=xTr[:, piece * 2 : (piece + 1) * 2,
                                c * 512 : (c + 1) * 512],
                    )
                    xts.append(xt)
                    if c == 0 and piece == 1:
                        nc.sync.dma_start(out=wq_sb[:, 0:8], in_=wqr[:, 0:8])
                    if c == 0 and piece == 3:
                        # second weight halves on SP (needed by k>=8 matmuls);
                        # constants ride the idle Pool queue
                        nc.sync.dma_start(out=wkv_sb[:, 8:16], in_=wkvr[:, 8:16])
                        nc.sync.dma_start(out=wq_sb[:, 8:16], in_=wqr[:, 8:16])
                        nc.gpsimd.dma_start(out=cos_sb, in_=cosb)
                        nc.gpsimd.dma_start(out=sin_sb, in_=sinb)
                        nc.gpsimd.dma_start(out=sinkv_st, in_=sink_v)
                        nc.vector.tensor_copy(
                            out=v_sb[0:SINK, NST - 1, 0:DH], in_=sinkv_st
                        )
                        nc.gpsimd.dma_start(out=sink_kT_sb, in_=sink_kT)
                        make_identity(nc, ident64)
                        make_identity(nc, identB)
                        make_identity(nc, idv[DH:128, :])
                        nc.vector.memset(v_sb[:, :, DH : DH + 1], 1.0)
                xt_tiles[c] = xts

            def proj_chunk(c):
                """Stream-major passes over prefetched x pieces."""
                cs = slice(c * 512, (c + 1) * 512)
                xts = xt_tiles.pop(c)
                kvps = psB.tile([128, 512], f32, tag="kvps")
                q01ps = psB.tile([128, 512], f32, tag="q01ps")
                q23ps = psB.tile([128, 512], f32, tag="q23ps")
                for k in range(16):
                    nc.tensor.matmul(
                        q01ps, lhsT=wq_sb[:, k, 0:128], rhs=xts[k // 2][:, k % 2, :],
                        start=(k == 0), stop=(k == 15),
                    )
                nc.vector.tensor_copy(out=q01[:, cs], in_=q01ps)
                rope_chunk(q01, cs, 128)
                nc.gpsimd.dma_start(out=qx1[:, cs], in_=q01[DH:128, cs])
                for k in range(16):
                    nc.tensor.matmul(
                        kvps, lhsT=wkv_sb[:, k, :], rhs=xts[k // 2][:, k % 2, :],
                        start=(k == 0), stop=(k == 15),
                    )
                nc.vector.tensor_copy(out=kk[:, cs], in_=kvps[0:DH, :])
                nc.vector.tensor_copy(out=vT64[DH:128, cs], in_=kvps[DH:128, :])
                rope_chunk(kk, cs, DH)
                if c < 3:
                    proj_dma(c + 1)
                for st in range(c * 4, c * 4 + 4):
                    vtps = psB.tile([128, DH], f32, tag="kvps")
                    nc.tensor.transpose(
                        vtps, vT64[DH:128, st * 128 : (st + 1) * 128],
                        idv[DH:128, :],
                    )
                    nc.vector.tensor_copy(out=v_sb[:, st, 0:DH], in_=vtps)
                for k in range(16):
                    nc.tensor.matmul(
                        q23ps, lhsT=wq_sb[:, k, 128:256], rhs=xts[k // 2][:, k % 2, :],
                        start=(k == 0), stop=(k == 15),
                    )
                nc.vector.tensor_copy(out=q23[:, cs], in_=q23ps)
                rope_chunk(q23, cs, 128)
                nc.gpsimd.dma_start(out=qx3[:, cs], in_=q23[DH:128, cs])

            # sink + c0-keys, u0-half only: feed ACT right after chunk 0
            EARLY = [(NST - 1, 0), (0, 0), (1, 0), (2, 0), (3, 0)]
            # the matching u1 halves + c1 keys (after chunk 1)
            FRONT2 = [(NST - 1, 1), (0, 1), (1, 1), (2, 1), (3, 1),
                      4, 5, 6, 7]
            BACK = list(range(8, 16))                   # keys from c2/c3
            proj_dma(0)
            # dummy matmuls span the initial DMA wait so the PE p-state ramp
            # is warm when the real stream arrives
            wps = psB.tile([128, 512], f32, tag="q01ps")
            for w in range(18):
                nc.tensor.matmul(wps[0:16, :], lhsT=warm[:, 0:16], rhs=warm,
                                 start=True, stop=True)
            proj_chunk(0)
            _interleave(scores_gen(0, EARLY))
            _interleave(scores_gen(1, EARLY))
            proj_chunk(1)
            _interleave(scores_gen(0, FRONT2))
            _interleave(scores_gen(1, FRONT2))
            proj_chunk(2)
            _interleave(scores_gen(0, [8, 9, 10, 11]), scores_gen(1, [8, 9, 10, 11]))
            proj_chunk(3)

        # ---- phase 2: software-pipelined attention + output projection ----
        with (
            tc.tile_pool(name="psC", bufs=1, space="PSUM") as psC,
            tc.tile_pool(name="psY", bufs=2, space="PSUM") as psY,
            tc.tile_pool(name="pLate", bufs=1) as pLate,
        ):
            wo_sb = pLate.tile([128, 2, DMODEL], bf16, tag="wo_sb")
            nc.gpsimd.dma_start(out=wo_sb, in_=wo.rearrange("(a p) n -> p a n", p=128))

            # R0: finish B0/B1 scores
            _interleave(scores_gen(0, [12, 13, 14, 15]), scores_gen(1, [12, 13, 14, 15]))
            # Rounds: sc(B_{k+2}) rides with ctx(B_k); wT bufs=3 keeps their
            # buffers distinct; norm(B_{k-1}) fully precedes ctx(B_k).
            _interleave(ctx_gen(0), (scores_gen(2, ST_ORDER), 1))
            for k in range(1, 6):
                _interleave(norm_gen(k - 1))
                _interleave(ctx_gen(k), (scores_gen(k + 2, ST_ORDER), 1))
            _interleave(norm_gen(5))
            _interleave(ctx_gen(6), (yout_gen(0, range(4)), 1))
            _interleave(norm_gen(6))
            _interleave(ctx_gen(7), (yout_gen(0, range(4, 8)), 1))
            _interleave(norm_gen(7, act_tail=True),
                        (yout_gen(1, range(8), act_tail=True), 2))

    nc.compile()
    return nc


def _host_inputs(x, kv_cache, Wq, Wk, Wv, Wo, start_pos):
    """Build the 8 per-core input dicts."""
    import ml_dtypes

    f32 = np.float32
    bf16 = ml_dtypes.bfloat16
    xT = np.ascontiguousarray(np.asarray(x, f32)[0].T.astype(bf16))  # (feat, tok)

    inv_freq = (1.0 / (10000.0 ** (np.arange(0, DH, 2, dtype=f32) / DH))).astype(f32)
    pos = np.arange(start_pos, start_pos + T, dtype=f32)
    ang = pos[:, None] * inv_freq[None, :]
    cosT = np.cos(ang).T.astype(f32)  # (32, T)
    sinT = np.sin(ang).T.astype(f32)
    cosb = np.ascontiguousarray(np.concatenate([cosT] * 4, axis=0).astype(bf16))
    sinb = np.ascontiguousarray(
        np.concatenate([-sinT, sinT, -sinT, sinT], axis=0).astype(bf16))

    Wq = np.asarray(Wq, f32)
    Wk = np.asarray(Wk, f32)
    Wv = np.asarray(Wv, f32)
    Wo = np.asarray(Wo, f32)
    kv_cache = np.asarray(kv_cache, f32)

    in_maps = []
    for i in range(NKV):
        sink = kv_cache[0, i, :SINK, :]
        sink_kT = np.ascontiguousarray(sink.T)
        in_maps.append(
            {
                "xT": xT,
                "wq": np.ascontiguousarray(
                    Wq[:, i * GROUP * DH : (i + 1) * GROUP * DH].astype(bf16)
                ),
                "wkv": np.ascontiguousarray(
                    np.concatenate(
                        [Wk[:, i * DH : (i + 1) * DH], Wv[:, i * DH : (i + 1) * DH]],
                        axis=1,
                    ).astype(bf16)
                ),
                "wo": np.ascontiguousarray(Wo[i * GROUP * DH : (i + 1) * GROUP * DH, :]),
                "sink_kT": sink_kT,
                "sink_v": np.ascontiguousarray(sink),
                "cosb": cosb,
                "sinb": sinb,
            }
        )
    return in_maps


def run(inputs, trace=False, trace_kwargs=None):
    """Run the 8-core kernel; returns (y, BassKernelResults)."""
    from concourse.bass_utils import run_bass_kernel_spmd

    if "nc" not in _CACHE:
        _CACHE["nc"] = _build_nc()
    nc = _CACHE["nc"]

    start_pos = int(np.asarray(inputs["start_pos"]))
    in_maps = _host_inputs(
        inputs["x"], inputs["kv_cache"], inputs["Wq"], inputs["Wk"], inputs["Wv"],
        inputs["Wo"], start_pos,
    )
    kwargs = {}
    if trace:
        kwargs["trace"] = True
        if trace_kwargs:
            kwargs["trace_kwargs"] = trace_kwargs
    res = run_bass_kernel_spmd(nc, in_maps, core_ids=list(range(NKV)), **kwargs)

    y = res.results[0]["out"].astype(np.float64)
    for i in range(1, NKV):
        y += res.results[i]["out"].astype(np.float64)
    y = (y + np.asarray(inputs["bo"], np.float64)[None, :]).astype(np.float32)
    return y[None], res


def kernel(**inputs):
    y, _ = run(inputs)
    return y


# revision 56
# speedup vs baseline: 1.0134x; 1.0134x over previous
"""GQA sparse-attention (sink + sliding window) kernel for 8 TRN2 NeuronCores.

Problem: nn_MultiHeadSelfAttentionModern (B=1, T=2048, D=2048, 32 q heads,
8 KV heads, d_head=64, WINDOW=2048, SINK=64, start_pos=2048, cache_len=2048).

Since S = cache_len + T = 4096 > WINDOW + SINK = 2112, the effective keys are
just kv_cache[:, :, :64] (the sink, used raw for both K and V) plus the 2048
new RoPE'd k (and raw new v).  Sharding: tensor-parallel by KV head - core i
owns KV head i and its 4 query heads, with Wq/Wk/Wv column-sharded and Wo
row-sharded; partial (bf16) outputs are summed on the host (+ bo).

Cost-model-optimized dataflow, software-pipelined in rounds:
  - projections in bf16 (x, Wq, Wkv host-cast) with k|v merged into one
    128-column psum stream; RoPE per 512-token chunk on DVE.
  - per (head, 1024-token half) block B: scores_T[s,t] on PE -> exp on ACT
    (the ~145us exp stream is the pacer); ctx is computed TRANSPOSED:
    ctx[t, d+1] += wT_st.T @ (v_st|ones) charges only 65 PE rows per matmul,
    accumulated tile-major (one open psum accumulation group per bank).
    The ones column yields softmax denominators per token-partition.
  - normalize = reciprocal + tensor_scalar_mul, PE-transpose back to [d, t]
    (odd heads hop to partitions 64-127 via one SBUF-SBUF DMA per block),
    y = ctxT.T @ Wo in psum, staged to SBUF (DVE, ACT at the tail) and
    DMA'd out as bf16.
"""

import numpy as np

T = 2048
DMODEL = 2048
NKV = 8
GROUP = 4
DH = 64
SINK = 64
NST = 17  # s-tiles: 16 full 128-tiles of new tokens + 1 sink tile (64 rows)
SCALE = 0.125  # 1/sqrt(64)

# ctx psum slot offsets (even-aligned: HW requires even psum element bases).
# 7 slots in bank A + 1 at the base of bank B; transpose staging at 640/768.
CTX_SLOTS = [0, 66, 132, 198, 264, 330, 396, 512]
TPS_SLOTS = [640, 768]
# ctx consumes s-tiles in the order their exps complete: sink first
ST_ORDER = [NST - 1] + list(range(16))

_CACHE = {}


def _interleave(*gens):
    """Round-robin the generators: one quantum each per cycle.

    Each entry is either a generator or (generator, start_delay_cycles).
    """
    slots = []
    for g in gens:
        if g is None:
            continue
        if isinstance(g, tuple):
            slots.append([g[0], g[1]])
        else:
            slots.append([g, 0])
    cycle = 0
    while slots:
        keep = []
        for ent in slots:
            g, delay = ent
            if cycle < delay:
                keep.append(ent)
                continue
            try:
                next(g)
                keep.append(ent)
            except StopIteration:
                pass
        slots = keep
        cycle += 1


def _build_nc():
    import concourse.bass as bass
    import concourse.mybir as mybir
    import concourse.tile as tile
    from concourse import bacc
    from concourse.masks import make_identity

    f32 = mybir.dt.float32
    f32r = mybir.dt.float32r
    bf16 = mybir.dt.bfloat16

    nc = bacc.Bacc("TRN2", target_bir_lowering=False, debug=False, num_devices=NKV)

    xT = nc.declare_dram_parameter("xT", [DMODEL, T], bf16, isOutput=False).ap()
    wq = nc.declare_dram_parameter("wq", [DMODEL, GROUP * DH], bf16, isOutput=False).ap()
    wkv = nc.declare_dram_parameter("wkv", [DMODEL, 2 * DH], bf16, isOutput=False).ap()
    wo = nc.declare_dram_parameter("wo", [GROUP * DH, DMODEL], f32, isOutput=False).ap()
    sink_kT = nc.declare_dram_parameter("sink_kT", [DH, SINK], f32r, isOutput=False).ap()
    sink_v = nc.declare_dram_parameter("sink_v", [SINK, DH], f32, isOutput=False).ap()
    cosb = nc.declare_dram_parameter("cosb", [128, T], bf16, isOutput=False).ap()
    sinb = nc.declare_dram_parameter("sinb", [128, T], bf16, isOutput=False).ap()
    out = nc.declare_dram_parameter("out", [T, DMODEL], bf16, isOutput=True).ap()

    # processing order of the 8 (head, half) blocks: half-major, odd
    # heads first so the final norms (gating yout) need no partition hop
    BLOCKS = [(1, 0), (3, 0), (0, 0), (2, 0), (1, 1), (3, 1), (0, 1), (2, 1)]

    with (
        tile.TileContext(nc) as tc,
        tc.tile_pool(name="persist", bufs=1) as persist,
        tc.tile_pool(name="psS", bufs=2, space="PSUM") as psS,
        tc.tile_pool(name="pm", bufs=1) as pm,
    ):
        q01 = persist.tile([128, T], f32r, tag="q01")
        q23 = persist.tile([128, T], f32r, tag="q23")
        qx1 = persist.tile([DH, T], f32r, tag="qx1")
        qx3 = persist.tile([DH, T], f32r, tag="qx3")
        kk = persist.tile([DH, T], f32r, tag="kk")
        v_sb = persist.tile([128, NST, DH + 1], bf16, tag="v_sb")
        ctxT = [persist.tile([128, T], bf16, tag=f"ctxT{j}", name=f"ctxT{j}") for j in range(2)]
        ident64 = persist.tile([SINK, SINK], f32, tag="ident64")
        identB = persist.tile([128, 128], f32, tag="identB")
        sink_kT_sb = persist.tile([DH, SINK], f32r, tag="sink_kT")
        cos_sb = persist.tile([128, T], bf16, tag="cos_sb")
        sin_sb = persist.tile([128, T], bf16, tag="sin_sb")
        recip_sb = persist.tile([128, 64], f32, tag="recip_sb")
        zero_sb = persist.tile([128, 1], f32, tag="zero_sb")
        nc.vector.memset(zero_sb, 0.0)

        xTr = xT.rearrange("(k p) t -> p k t", p=128)
        wkvr = wkv.rearrange("(k p) m -> p k m", p=128)
        wqr = wq.rearrange("(k p) m -> p k m", p=128)

        qsrc = [q01[0:DH, :], qx1, q23[0:DH, :], qx3]
        wT_tiles = {}
        cps_tiles = {}

        def get_wT(bi):
            if bi not in wT_tiles:
                h, half = BLOCKS[bi]
                wT_tiles[bi] = pm.tile(
                    [128, NST, 1024], bf16, tag="wT", bufs=3, name=f"wT{h}_{half}"
                )
            return wT_tiles[bi]

        def scores_gen(bi, sts):
            """One quantum per item: item = st or (st, u-half)."""
            h, half = BLOCKS[bi]
            wT = get_wT(bi)
            qt = qsrc[h]
            c0 = half * 1024
            for item in sts:
                st, uh = item if isinstance(item, tuple) else (item, None)
                p = SINK if st == NST - 1 else 128
                sps = psS.tile([128, 1024], f32, tag="sps", name="sps")
                lhsT = sink_kT_sb if st == NST - 1 else kk[:, st * 128 : (st + 1) * 128]
                us = range(2) if uh is None else (uh,)
                for u in us:
                    nc.tensor.matmul(
                        sps[0:p, u * 512 : (u + 1) * 512],
                        lhsT=lhsT,
                        rhs=qt[:, c0 + u * 512 : c0 + (u + 1) * 512],
                        start=True,
                        stop=True,
                    )
                if uh is None:
                    nc.scalar.activation(
                        out=wT[0:p, st, :],
                        in_=sps[0:p, :],
                        func=mybir.ActivationFunctionType.Exp,
                        bias=zero_sb[0:p, :],
                        scale=SCALE,
                    )
                else:
                    nc.scalar.activation(
                        out=wT[0:p, st, uh * 512 : (uh + 1) * 512],
                        in_=sps[0:p, uh * 512 : (uh + 1) * 512],
                        func=mybir.ActivationFunctionType.Exp,
                        bias=zero_sb[0:p, :],
                        scale=SCALE,
                    )
                yield

        def ctx_gen(bi):
            """tl-major ctx accumulation: 17 matmuls per token-tile slot
            (one open psum accumulation group per bank region at a time)."""
            h, half = BLOCKS[bi]
            wT = get_wT(bi)
            cps = psC.tile([128, 1024], f32, tag="cps", name=f"cps{h}_{half}")
            cps_tiles[bi] = (cps, [(cps, CTX_SLOTS[tl]) for tl in range(8)])
            for tl in range(8):
                for si, st in enumerate(ST_ORDER):
                    p = SINK if st == NST - 1 else 128
                    nc.tensor.matmul(
                        cps[:, CTX_SLOTS[tl] : CTX_SLOTS[tl] + DH + 1],
                        lhsT=wT[0:p, st, tl * 128 : (tl + 1) * 128],
                        rhs=v_sb[0:p, st, :],
                        start=(si == 0),
                        stop=(si == NST - 1),
                    )
                yield

        def norm_gen(bi, act_tail=False, per_slot=False):
            """Normalize + transpose ctx back to [d, t] bf16 in ctxT."""
            h, half = BLOCKS[bi]
            cps, slots = cps_tiles.pop(bi)
            roff = h * 16 + half * 8
            if per_slot:
                # per-slot recips: each tile's normalize becomes ready right
                # after its own ctx slot instead of after all eight
                for tl, (tile_, off) in enumerate(slots):
                    nc.vector.reciprocal(
                        recip_sb[:, roff + tl : roff + tl + 1],
                        tile_[:, off + DH : off + DH + 1],
                    )
            else:
                row = cps[:, DH : DH + 1]
                sums_a = bass.AP(
                    tensor=row.tensor, offset=row.offset,
                    ap=[row.ap[0]] + [[66, 7]],
                )
                nc.vector.reciprocal(recip_sb[:, roff : roff + 7], sums_a)
                nc.vector.reciprocal(
                    recip_sb[:, roff + 7 : roff + 8],
                    cps[:, CTX_SLOTS[7] + DH : CTX_SLOTS[7] + DH + 1],
                )
            yield
            codd = None
            if h % 2 == 1:
                codd = pm.tile([DH, 1024], bf16, tag="codd", bufs=2, name="codd")
            for tl in range(8):
                stile, slot = slots[tl]
                tt = half * 8 + tl
                ctxn = pm.tile([128, DH], f32, tag="ctxn", bufs=4, name="ctxn")
                nc.vector.tensor_scalar_mul(
                    out=ctxn,
                    in0=stile[:, slot : slot + DH],
                    scalar1=recip_sb[:, roff + tl : roff + tl + 1],
                )
                tslot = TPS_SLOTS[tl % 2]
                nc.tensor.transpose(cps[0:DH, tslot : tslot + 128], ctxn, identB)
                if h % 2 == 0:
                    nc.vector.tensor_copy(
                        out=ctxT[h // 2][0:DH, tt * 128 : (tt + 1) * 128],
                        in_=cps[0:DH, tslot : tslot + 128],
                    )
                elif act_tail:
                    nc.scalar.copy(
                        out=codd[:, tl * 128 : (tl + 1) * 128],
                        in_=cps[0:DH, tslot : tslot + 128],
                    )
                else:
                    nc.vector.tensor_copy(
                        out=codd[:, tl * 128 : (tl + 1) * 128],
                        in_=cps[0:DH, tslot : tslot + 128],
                    )
                if tl % 2 == 1:
                    if h % 2 == 1:
                        # lane-aligned engines can't shift partitions; DMA
                        # hops each finished pair to partitions 64-127
                        nc.gpsimd.dma_start(
                            out=ctxT[h // 2][
                                DH:128,
                                half * 1024 + (tl - 1) * 128
                                : half * 1024 + (tl + 1) * 128,
                            ],
                            in_=codd[:, (tl - 1) * 128 : (tl + 1) * 128],
                        )
                    yield
            yield

        def yout_gen(half, tls, act_tail=False):
            for tl in tls:
                tt = half * 8 + tl
                y_sb = pLate.tile([128, DMODEL], bf16, tag="y_sb", bufs=3,
                                  name="y_sb")
                for nck in range(4):
                    yps = psY.tile([128, 512], f32, tag="yps", name="yps")
                    for j in range(2):
                        nc.tensor.matmul(
                            yps,
                            lhsT=ctxT[j][:, tt * 128 : (tt + 1) * 128],
                            rhs=wo_sb[:, j, nck * 512 : (nck + 1) * 512],
                            start=(j == 0),
                            stop=(j == 1),
                        )
                    dst = y_sb[:, nck * 512 : (nck + 1) * 512]
                    if act_tail and nck % 2 == 1:
                        # ACT's exp stream is over; share the tail copies
                        nc.scalar.copy(out=dst, in_=yps)
                    else:
                        nc.vector.tensor_copy(out=dst, in_=yps)
                    if nck % 2 == 1:
                        yield
                nc.sync.dma_start(
                    out=out[tt * 128 : (tt + 1) * 128, :], in_=y_sb
                )

        # ---- phase 1: projections + per-chunk rope + v transpose ----
        with (
            tc.tile_pool(name="psB", bufs=1, space="PSUM") as psB,
            tc.tile_pool(name="pw", bufs=1) as pw,
            tc.tile_pool(name="px", bufs=2) as px,
        ):
            wq_sb = pw.tile([128, 16, GROUP * DH], bf16, tag="wq_sb")
            wkv_sb = pw.tile([128, 16, 2 * DH], bf16, tag="wkv_sb")
            # v^T staged on partitions 64-127 (straight DVE copy from the kv
            # psum stream); idv holds a 64x64 identity on partitions 64-127
            vT64 = pw.tile([128, T], f32, tag="vT64")
            idv = pw.tile([128, DH], f32, tag="idv")
            sinkv_st = pw.tile([SINK, DH], f32, tag="sinkv_st")
            warm = px.tile([128, 512], bf16, tag="warm")
            nc.vector.memset(warm, 0.25)
            nc.sync.dma_start(out=wkv_sb[:, 0:8], in_=wkvr[:, 0:8])

            def rope_chunk(tgt, cs, pp, eng=None):
                """tgt[0:pp, cs] <- tgt*C + swap32(tgt)*S on token slice cs."""
                eng = eng or nc.gpsimd
                sw = px.tile([128, 512], f32r, tag="sw", bufs=2)
                n = cs.stop - cs.start
                for b in range(pp // 64):
                    eng.dma_start(
                        out=sw[b * 64 : b * 64 + 32, 0:n],
                        in_=tgt[b * 64 + 32 : b * 64 + 64, cs],
                    )
                    eng.dma_start(
                        out=sw[b * 64 + 32 : b * 64 + 64, 0:n],
                        in_=tgt[b * 64 : b * 64 + 32, cs],
                    )
                nc.vector.tensor_mul(sw[0:pp, 0:n], sw[0:pp, 0:n], sin_sb[0:pp, cs])
                nc.vector.tensor_mul(tgt[0:pp, cs], tgt[0:pp, cs], cos_sb[0:pp, cs])
                nc.vector.tensor_add(tgt[0:pp, cs], tgt[0:pp, cs], sw[0:pp, 0:n])

            xt_tiles = {}

            def proj_dma(c):
                """Queue the 8 x-piece loads of chunk c (cross-chunk prefetch)."""
                xts = []
                for piece in range(8):
                    xt = px.tile([128, 2, 512], bf16, tag="xt", bufs=7)
                    nc.sync.dma_start(
                        out=xt,
                        in_=xTr[:, piece * 2 : (piece + 1) * 2,
                                c * 512 : (c + 1) * 512],
                    )
                    xts.append(xt)
                    if c == 0 and piece == 1:
                        nc.sync.dma_start(out=wq_sb[:, 0:8], in_=wqr[:, 0:8])
                    if c == 0 and piece == 3:
                        # second weight halves on SP (needed by k>=8 matmuls);
                        # constants ride the idle Pool queue
                        nc.sync.dma_start(out=wkv_sb[:, 8:16], in_=wkvr[:, 8:16])
                        nc.sync.dma_start(out=wq_sb[:, 8:16], in_=wqr[:, 8:16])
                        nc.gpsimd.dma_start(out=cos_sb, in_=cosb)
                        nc.gpsimd.dma_start(out=sin_sb, in_=sinb)
                        nc.gpsimd.dma_start(out=sinkv_st, in_=sink_v)
                        nc.vector.tensor_copy(
                            out=v_sb[0:SINK, NST - 1, 0:DH], in_=sinkv_st
                        )
                        nc.gpsimd.dma_start(out=sink_kT_sb, in_=sink_kT)
                        make_identity(nc, ident64)
                        make_identity(nc, identB)
                        make_identity(nc, idv[DH:128, :])
                        nc.vector.memset(v_sb[:, :, DH : DH + 1], 1.0)
                xt_tiles[c] = xts

            def proj_chunk(c):
                """Stream-major passes over prefetched x pieces."""
                cs = slice(c * 512, (c + 1) * 512)
                xts = xt_tiles.pop(c)
                kvps = psB.tile([128, 512], f32, tag="kvps")
                q01ps = psB.tile([128, 512], f32, tag="q01ps")
                q23ps = psB.tile([128, 512], f32, tag="q23ps")
                for k in range(16):
                    nc.tensor.matmul(
                        q01ps, lhsT=wq_sb[:, k, 0:128], rhs=xts[k // 2][:, k % 2, :],
                        start=(k == 0), stop=(k == 15),
                    )
                nc.vector.tensor_copy(out=q01[:, cs], in_=q01ps)
                rope_chunk(q01, cs, 128)
                nc.gpsimd.dma_start(out=qx1[:, cs], in_=q01[DH:128, cs])
                for k in range(16):
                    nc.tensor.matmul(
                        kvps, lhsT=wkv_sb[:, k, :], rhs=xts[k // 2][:, k % 2, :],
                        start=(k == 0), stop=(k == 15),
                    )
                nc.vector.tensor_copy(out=kk[:, cs], in_=kvps[0:DH, :])
                nc.vector.tensor_copy(out=vT64[DH:128, cs], in_=kvps[DH:128, :])
                rope_chunk(kk, cs, DH)
                if c < 3:
                    proj_dma(c + 1)
                for st in range(c * 4, c * 4 + 4):
                    vtps = psB.tile([128, DH], f32, tag="kvps")
                    nc.tensor.transpose(
                        vtps, vT64[DH:128, st * 128 : (st + 1) * 128],
                        idv[DH:128, :],
                    )
                    nc.vector.tensor_copy(out=v_sb[:, st, 0:DH], in_=vtps)
                for k in range(16):
                    nc.tensor.matmul(
                        q23ps, lhsT=wq_sb[:, k, 128:256], rhs=xts[k // 2][:, k % 2, :],
                        start=(k == 0), stop=(k == 15),
                    )
                nc.vector.tensor_copy(out=q23[:, cs], in_=q23ps)
                rope_chunk(q23, cs, 128)
                nc.gpsimd.dma_start(out=qx3[:, cs], in_=q23[DH:128, cs])

            # sink + c0-keys, u0-half only: feed ACT right after chunk 0
            EARLY = [(NST - 1, 0), (0, 0), (1, 0), (2, 0), (3, 0)]
            # the matching u1 halves + c1 keys (after chunk 1)
            FRONT2 = [(NST - 1, 1), (0, 1), (1, 1), (2, 1), (3, 1),
                      4, 5, 6, 7]
            BACK = list(range(8, 16))                   # keys from c2/c3
            proj_dma(0)
            # dummy matmuls span the initial DMA wait so the PE p-state ramp
            # is warm when the real stream arrives
            wps = psB.tile([128, 512], f32, tag="q01ps")
            for w in range(18):
                nc.tensor.matmul(wps[0:16, :], lhsT=warm[:, 0:16], rhs=warm,
                                 start=True, stop=True)
            proj_chunk(0)
            _interleave(scores_gen(0, EARLY))
            _interleave(scores_gen(1, EARLY))
            proj_chunk(1)
            _interleave(scores_gen(0, FRONT2))
            _interleave(scores_gen(1, FRONT2))
            proj_chunk(2)
            _interleave(scores_gen(0, [8, 9, 10, 11]), scores_gen(1, [8, 9, 10, 11]))
            proj_chunk(3)

        # ---- phase 2: software-pipelined attention + output projection ----
        with (
            tc.tile_pool(name="psC", bufs=1, space="PSUM") as psC,
            tc.tile_pool(name="psY", bufs=2, space="PSUM") as psY,
            tc.tile_pool(name="pLate", bufs=1) as pLate,
        ):
            wo_sb = pLate.tile([128, 2, DMODEL], bf16, tag="wo_sb")
            nc.gpsimd.dma_start(out=wo_sb, in_=wo.rearrange("(a p) n -> p a n", p=128))

            # R0: finish B0/B1 scores
            _interleave(scores_gen(0, [12, 13, 14, 15]), scores_gen(1, [12, 13, 14, 15]))
            # Rounds: sc(B_{k+2}) rides with ctx(B_k); wT bufs=3 keeps their
            # buffers distinct; norm(B_{k-1}) fully precedes ctx(B_k).
            _interleave(ctx_gen(0), (scores_gen(2, ST_ORDER), 1))
            for k in range(1, 6):
                _interleave(norm_gen(k - 1))
                _interleave(ctx_gen(k), (scores_gen(k + 2, ST_ORDER), 1))
            _interleave(norm_gen(5))
            _interleave(ctx_gen(6), (yout_gen(0, range(4)), 1))
            _interleave(norm_gen(6))
            _interleave(ctx_gen(7), (yout_gen(0, range(4, 8)), 1))
            _interleave(norm_gen(7, act_tail=True),
                        (yout_gen(1, range(8), act_tail=True), 2))

    nc.compile()
    return nc


def _host_inputs(x, kv_cache, Wq, Wk, Wv, Wo, start_pos):
    """Build the 8 per-core input dicts."""
    import ml_dtypes

    f32 = np.float32
    bf16 = ml_dtypes.bfloat16
    xT = np.ascontiguousarray(np.asarray(x, f32)[0].T.astype(bf16))  # (feat, tok)

    inv_freq = (1.0 / (10000.0 ** (np.arange(0, DH, 2, dtype=f32) / DH))).astype(f32)
    pos = np.arange(start_pos, start_pos + T, dtype=f32)
    ang = pos[:, None] * inv_freq[None, :]
    cosT = np.cos(ang).T.astype(f32)  # (32, T)
    sinT = np.sin(ang).T.astype(f32)
    cosb = np.ascontiguousarray(np.concatenate([cosT] * 4, axis=0).astype(bf16))
    sinb = np.ascontiguousarray(
        np.concatenate([-sinT, sinT, -sinT, sinT], axis=0).astype(bf16))

    Wq = np.asarray(Wq, f32)
    Wk = np.asarray(Wk, f32)
    Wv = np.asarray(Wv, f32)
    Wo = np.asarray(Wo, f32)
    kv_cache = np.asarray(kv_cache, f32)

    in_maps = []
    for i in range(NKV):
        sink = kv_cache[0, i, :SINK, :]
        sink_kT = np.ascontiguousarray(sink.T)
        in_maps.append(
            {
                "xT": xT,
                "wq": np.ascontiguousarray(
                    Wq[:, i * GROUP * DH : (i + 1) * GROUP * DH].astype(bf16)
                ),
                "wkv": np.ascontiguousarray(
                    np.concatenate(
                        [Wk[:, i * DH : (i + 1) * DH], Wv[:, i * DH : (i + 1) * DH]],
                        axis=1,
                    ).astype(bf16)
                ),
                "wo": np.ascontiguousarray(Wo[i * GROUP * DH : (i + 1) * GROUP * DH, :]),
                "sink_kT": sink_kT,
                "sink_v": np.ascontiguousarray(sink),
                "cosb": cosb,
                "sinb": sinb,
            }
        )
    return in_maps


def run(inputs, trace=False, trace_kwargs=None):
    """Run the 8-core kernel; returns (y, BassKernelResults)."""
    from concourse.bass_utils import run_bass_kernel_spmd

    if "nc" not in _CACHE:
        _CACHE["nc"] = _build_nc()
    nc = _CACHE["nc"]

    start_pos = int(np.asarray(inputs["start_pos"]))
    in_maps = _host_inputs(
        inputs["x"], inputs["kv_cache"], inputs["Wq"], inputs["Wk"], inputs["Wv"],
        inputs["Wo"], start_pos,
    )
    kwargs = {}
    if trace:
        kwargs["trace"] = True
        if trace_kwargs:
            kwargs["trace_kwargs"] = trace_kwargs
    res = run_bass_kernel_spmd(nc, in_maps, core_ids=list(range(NKV)), **kwargs)

    y = res.results[0]["out"].astype(np.float64)
    for i in range(1, NKV):
        y += res.results[i]["out"].astype(np.float64)
    y = (y + np.asarray(inputs["bo"], np.float64)[None, :]).astype(np.float32)
    return y[None], res


def kernel(**inputs):
    y, _ = run(inputs)
    return y


# revision 57
# speedup vs baseline: 1.0138x; 1.0003x over previous
"""GQA sparse-attention (sink + sliding window) kernel for 8 TRN2 NeuronCores.

Problem: nn_MultiHeadSelfAttentionModern (B=1, T=2048, D=2048, 32 q heads,
8 KV heads, d_head=64, WINDOW=2048, SINK=64, start_pos=2048, cache_len=2048).

Since S = cache_len + T = 4096 > WINDOW + SINK = 2112, the effective keys are
just kv_cache[:, :, :64] (the sink, used raw for both K and V) plus the 2048
new RoPE'd k (and raw new v).  Sharding: tensor-parallel by KV head - core i
owns KV head i and its 4 query heads, with Wq/Wk/Wv column-sharded and Wo
row-sharded; partial (bf16) outputs are summed on the host (+ bo).

Cost-model-optimized dataflow, software-pipelined in rounds:
  - projections in bf16 (x, Wq, Wkv host-cast) with k|v merged into one
    128-column psum stream; RoPE per 512-token chunk on DVE.
  - per (head, 1024-token half) block B: scores_T[s,t] on PE -> exp on ACT
    (the ~145us exp stream is the pacer); ctx is computed TRANSPOSED:
    ctx[t, d+1] += wT_st.T @ (v_st|ones) charges only 65 PE rows per matmul,
    accumulated tile-major (one open psum accumulation group per bank).
    The ones column yields softmax denominators per token-partition.
  - normalize = reciprocal + tensor_scalar_mul, PE-transpose back to [d, t]
    (odd heads hop to partitions 64-127 via one SBUF-SBUF DMA per block),
    y = ctxT.T @ Wo in psum, staged to SBUF (DVE, ACT at the tail) and
    DMA'd out as bf16.
"""

import numpy as np

T = 2048
DMODEL = 2048
NKV = 8
GROUP = 4
DH = 64
SINK = 64
NST = 17  # s-tiles: 16 full 128-tiles of new tokens + 1 sink tile (64 rows)
SCALE = 0.125  # 1/sqrt(64)

# ctx psum slot offsets (even-aligned: HW requires even psum element bases).
# 7 slots in bank A + 1 at the base of bank B; transpose staging at 640/768.
CTX_SLOTS = [0, 66, 132, 198, 264, 330, 396, 512]
TPS_SLOTS = [640, 768]
# ctx consumes s-tiles in the order their exps complete: sink first
ST_ORDER = [NST - 1] + list(range(16))

_CACHE = {}


def _interleave(*gens):
    """Round-robin the generators: one quantum each per cycle.

    Each entry is either a generator or (generator, start_delay_cycles).
    """
    slots = []
    for g in gens:
        if g is None:
            continue
        if isinstance(g, tuple):
            slots.append([g[0], g[1]])
        else:
            slots.append([g, 0])
    cycle = 0
    while slots:
        keep = []
        for ent in slots:
            g, delay = ent
            if cycle < delay:
                keep.append(ent)
                continue
            try:
                next(g)
                keep.append(ent)
            except StopIteration:
                pass
        slots = keep
        cycle += 1


def _build_nc():
    import concourse.bass as bass
    import concourse.mybir as mybir
    import concourse.tile as tile
    from concourse import bacc
    from concourse.masks import make_identity

    f32 = mybir.dt.float32
    f32r = mybir.dt.float32r
    bf16 = mybir.dt.bfloat16

    nc = bacc.Bacc("TRN2", target_bir_lowering=False, debug=False, num_devices=NKV)

    xT = nc.declare_dram_parameter("xT", [DMODEL, T], bf16, isOutput=False).ap()
    wq = nc.declare_dram_parameter("wq", [DMODEL, GROUP * DH], bf16, isOutput=False).ap()
    wkv = nc.declare_dram_parameter("wkv", [DMODEL, 2 * DH], bf16, isOutput=False).ap()
    wo = nc.declare_dram_parameter("wo", [GROUP * DH, DMODEL], f32, isOutput=False).ap()
    sink_kT = nc.declare_dram_parameter("sink_kT", [DH, SINK], f32r, isOutput=False).ap()
    sink_v = nc.declare_dram_parameter("sink_v", [SINK, DH], f32, isOutput=False).ap()
    cosb = nc.declare_dram_parameter("cosb", [128, T], bf16, isOutput=False).ap()
    sinb = nc.declare_dram_parameter("sinb", [128, T], bf16, isOutput=False).ap()
    out = nc.declare_dram_parameter("out", [T, DMODEL], bf16, isOutput=True).ap()

    # processing order of the 8 (head, half) blocks: half-major, odd
    # heads first so the final norms (gating yout) need no partition hop
    BLOCKS = [(1, 0), (3, 0), (0, 0), (2, 0), (1, 1), (3, 1), (0, 1), (2, 1)]

    with (
        tile.TileContext(nc) as tc,
        tc.tile_pool(name="persist", bufs=1) as persist,
        tc.tile_pool(name="psS", bufs=2, space="PSUM") as psS,
        tc.tile_pool(name="pm", bufs=1) as pm,
    ):
        q01 = persist.tile([128, T], f32r, tag="q01")
        q23 = persist.tile([128, T], f32r, tag="q23")
        qx1 = persist.tile([DH, T], f32r, tag="qx1")
        qx3 = persist.tile([DH, T], f32r, tag="qx3")
        kk = persist.tile([DH, T], f32r, tag="kk")
        v_sb = persist.tile([128, NST, DH + 1], bf16, tag="v_sb")
        ctxT = [persist.tile([128, T], bf16, tag=f"ctxT{j}", name=f"ctxT{j}") for j in range(2)]
        ident64 = persist.tile([SINK, SINK], f32, tag="ident64")
        identB = persist.tile([128, 128], f32, tag="identB")
        sink_kT_sb = persist.tile([DH, SINK], f32r, tag="sink_kT")
        cos_sb = persist.tile([128, T], bf16, tag="cos_sb")
        sin_sb = persist.tile([128, T], bf16, tag="sin_sb")
        recip_sb = persist.tile([128, 64], f32, tag="recip_sb")
        zero_sb = persist.tile([128, 1], f32, tag="zero_sb")
        nc.vector.memset(zero_sb, 0.0)

        xTr = xT.rearrange("(k p) t -> p k t", p=128)
        wkvr = wkv.rearrange("(k p) m -> p k m", p=128)
        wqr = wq.rearrange("(k p) m -> p k m", p=128)

        qsrc = [q01[0:DH, :], qx1, q23[0:DH, :], qx3]
        wT_tiles = {}
        cps_tiles = {}

        def get_wT(bi):
            if bi not in wT_tiles:
                h, half = BLOCKS[bi]
                wT_tiles[bi] = pm.tile(
                    [128, NST, 1024], bf16, tag="wT", bufs=3, name=f"wT{h}_{half}"
                )
            return wT_tiles[bi]

        def scores_gen(bi, sts):
            """One quantum per item: item = st or (st, u-half)."""
            h, half = BLOCKS[bi]
            wT = get_wT(bi)
            qt = qsrc[h]
            c0 = half * 1024
            for item in sts:
                st, uh = item if isinstance(item, tuple) else (item, None)
                p = SINK if st == NST - 1 else 128
                sps = psS.tile([128, 1024], f32, tag="sps", name="sps")
                lhsT = sink_kT_sb if st == NST - 1 else kk[:, st * 128 : (st + 1) * 128]
                us = range(2) if uh is None else (uh,)
                for u in us:
                    nc.tensor.matmul(
                        sps[0:p, u * 512 : (u + 1) * 512],
                        lhsT=lhsT,
                        rhs=qt[:, c0 + u * 512 : c0 + (u + 1) * 512],
                        start=True,
                        stop=True,
                    )
                if uh is None:
                    nc.scalar.activation(
                        out=wT[0:p, st, :],
                        in_=sps[0:p, :],
                        func=mybir.ActivationFunctionType.Exp,
                        bias=zero_sb[0:p, :],
                        scale=SCALE,
                    )
                else:
                    nc.scalar.activation(
                        out=wT[0:p, st, uh * 512 : (uh + 1) * 512],
                        in_=sps[0:p, uh * 512 : (uh + 1) * 512],
                        func=mybir.ActivationFunctionType.Exp,
                        bias=zero_sb[0:p, :],
                        scale=SCALE,
                    )
                yield

        def ctx_gen(bi):
            """tl-major ctx accumulation: 17 matmuls per token-tile slot
            (one open psum accumulation group per bank region at a time)."""
            h, half = BLOCKS[bi]
            wT = get_wT(bi)
            cps = psC.tile([128, 1024], f32, tag="cps", name=f"cps{h}_{half}")
            cps_tiles[bi] = (cps, [(cps, CTX_SLOTS[tl]) for tl in range(8)])
            for tl in range(8):
                for si, st in enumerate(ST_ORDER):
                    p = SINK if st == NST - 1 else 128
                    nc.tensor.matmul(
                        cps[:, CTX_SLOTS[tl] : CTX_SLOTS[tl] + DH + 1],
                        lhsT=wT[0:p, st, tl * 128 : (tl + 1) * 128],
                        rhs=v_sb[0:p, st, :],
                        start=(si == 0),
                        stop=(si == NST - 1),
                    )
                yield

        def norm_gen(bi, act_tail=False, per_slot=False):
            """Normalize + transpose ctx back to [d, t] bf16 in ctxT."""
            h, half = BLOCKS[bi]
            cps, slots = cps_tiles.pop(bi)
            roff = h * 16 + half * 8
            if per_slot:
                # per-slot recips: each tile's normalize becomes ready right
                # after its own ctx slot instead of after all eight
                for tl, (tile_, off) in enumerate(slots):
                    nc.vector.reciprocal(
                        recip_sb[:, roff + tl : roff + tl + 1],
                        tile_[:, off + DH : off + DH + 1],
                    )
            else:
                row = cps[:, DH : DH + 1]
                sums_a = bass.AP(
                    tensor=row.tensor, offset=row.offset,
                    ap=[row.ap[0]] + [[66, 7]],
                )
                nc.vector.reciprocal(recip_sb[:, roff : roff + 7], sums_a)
                nc.vector.reciprocal(
                    recip_sb[:, roff + 7 : roff + 8],
                    cps[:, CTX_SLOTS[7] + DH : CTX_SLOTS[7] + DH + 1],
                )
            yield
            codd = None
            if h % 2 == 1:
                codd = pm.tile([DH, 1024], bf16, tag="codd", bufs=2, name="codd")
            for tl in range(8):
                stile, slot = slots[tl]
                tt = half * 8 + tl
                ctxn = pm.tile([128, DH], f32, tag="ctxn", bufs=4, name="ctxn")
                nc.vector.tensor_scalar_mul(
                    out=ctxn,
                    in0=stile[:, slot : slot + DH],
                    scalar1=recip_sb[:, roff + tl : roff + tl + 1],
                )
                tslot = TPS_SLOTS[tl % 2]
                nc.tensor.transpose(cps[0:DH, tslot : tslot + 128], ctxn, identB)
                if h % 2 == 0:
                    nc.vector.tensor_copy(
                        out=ctxT[h // 2][0:DH, tt * 128 : (tt + 1) * 128],
                        in_=cps[0:DH, tslot : tslot + 128],
                    )
                elif act_tail:
                    nc.scalar.copy(
                        out=codd[:, tl * 128 : (tl + 1) * 128],
                        in_=cps[0:DH, tslot : tslot + 128],
                    )
                else:
                    nc.vector.tensor_copy(
                        out=codd[:, tl * 128 : (tl + 1) * 128],
                        in_=cps[0:DH, tslot : tslot + 128],
                    )
                if tl % 2 == 1:
                    if h % 2 == 1:
                        # lane-aligned engines can't shift partitions; DMA
                        # hops each finished pair to partitions 64-127
                        nc.gpsimd.dma_start(
                            out=ctxT[h // 2][
                                DH:128,
                                half * 1024 + (tl - 1) * 128
                                : half * 1024 + (tl + 1) * 128,
                            ],
                            in_=codd[:, (tl - 1) * 128 : (tl + 1) * 128],
                        )
                    yield
            yield

        def yout_gen(half, tls, act_tail=False):
            for tl in tls:
                tt = half * 8 + tl
                y_sb = pLate.tile([128, DMODEL], bf16, tag="y_sb", bufs=4,
                                  name="y_sb")
                for nck in range(4):
                    yps = psY.tile([128, 512], f32, tag="yps", name="yps")
                    for j in range(2):
                        nc.tensor.matmul(
                            yps,
                            lhsT=ctxT[j][:, tt * 128 : (tt + 1) * 128],
                            rhs=wo_sb[:, j, nck * 512 : (nck + 1) * 512],
                            start=(j == 0),
                            stop=(j == 1),
                        )
                    dst = y_sb[:, nck * 512 : (nck + 1) * 512]
                    if act_tail and nck % 2 == 1:
                        # ACT's exp stream is over; share the tail copies
                        nc.scalar.copy(out=dst, in_=yps)
                    else:
                        nc.vector.tensor_copy(out=dst, in_=yps)
                    if nck % 2 == 1:
                        yield
                nc.sync.dma_start(
                    out=out[tt * 128 : (tt + 1) * 128, :], in_=y_sb
                )

        # ---- phase 1: projections + per-chunk rope + v transpose ----
        with (
            tc.tile_pool(name="psB", bufs=1, space="PSUM") as psB,
            tc.tile_pool(name="pw", bufs=1) as pw,
            tc.tile_pool(name="px", bufs=2) as px,
        ):
            wq_sb = pw.tile([128, 16, GROUP * DH], bf16, tag="wq_sb")
            wkv_sb = pw.tile([128, 16, 2 * DH], bf16, tag="wkv_sb")
            # v^T staged on partitions 64-127 (straight DVE copy from the kv
            # psum stream); idv holds a 64x64 identity on partitions 64-127
            vT64 = pw.tile([128, T], f32, tag="vT64")
            idv = pw.tile([128, DH], f32, tag="idv")
            sinkv_st = pw.tile([SINK, DH], f32, tag="sinkv_st")
            warm = px.tile([128, 512], bf16, tag="warm")
            nc.vector.memset(warm, 0.25)
            nc.sync.dma_start(out=wkv_sb[:, 0:8], in_=wkvr[:, 0:8])

            def rope_chunk(tgt, cs, pp, eng=None):
                """tgt[0:pp, cs] <- tgt*C + swap32(tgt)*S on token slice cs."""
                eng = eng or nc.gpsimd
                sw = px.tile([128, 512], f32r, tag="sw", bufs=2)
                n = cs.stop - cs.start
                for b in range(pp // 64):
                    eng.dma_start(
                        out=sw[b * 64 : b * 64 + 32, 0:n],
                        in_=tgt[b * 64 + 32 : b * 64 + 64, cs],
                    )
                    eng.dma_start(
                        out=sw[b * 64 + 32 : b * 64 + 64, 0:n],
                        in_=tgt[b * 64 : b * 64 + 32, cs],
                    )
                nc.vector.tensor_mul(sw[0:pp, 0:n], sw[0:pp, 0:n], sin_sb[0:pp, cs])
                nc.vector.tensor_mul(tgt[0:pp, cs], tgt[0:pp, cs], cos_sb[0:pp, cs])
                nc.vector.tensor_add(tgt[0:pp, cs], tgt[0:pp, cs], sw[0:pp, 0:n])

            xt_tiles = {}

            def proj_dma(c):
                """Queue the 8 x-piece loads of chunk c (cross-chunk prefetch)."""
                xts = []
                for piece in range(8):
                    xt = px.tile([128, 2, 512], bf16, tag="xt", bufs=7)
                    nc.sync.dma_start(
                        out=xt,
                        in_=xTr[:, piece * 2 : (piece + 1) * 2,
                                c * 512 : (c + 1) * 512],
                    )
                    xts.append(xt)
                    if c == 0 and piece == 1:
                        nc.sync.dma_start(out=wq_sb[:, 0:8], in_=wqr[:, 0:8])
                    if c == 0 and piece == 3:
                        # second weight halves on SP (needed by k>=8 matmuls);
                        # constants ride the idle Pool queue
                        nc.sync.dma_start(out=wkv_sb[:, 8:16], in_=wkvr[:, 8:16])
                        nc.sync.dma_start(out=wq_sb[:, 8:16], in_=wqr[:, 8:16])
                        nc.gpsimd.dma_start(out=cos_sb, in_=cosb)
                        nc.gpsimd.dma_start(out=sin_sb, in_=sinb)
                        nc.gpsimd.dma_start(out=sinkv_st, in_=sink_v)
                        nc.vector.tensor_copy(
                            out=v_sb[0:SINK, NST - 1, 0:DH], in_=sinkv_st
                        )
                        nc.gpsimd.dma_start(out=sink_kT_sb, in_=sink_kT)
                        make_identity(nc, ident64)
                        make_identity(nc, identB)
                        make_identity(nc, idv[DH:128, :])
                        nc.vector.memset(v_sb[:, :, DH : DH + 1], 1.0)
                xt_tiles[c] = xts

            def proj_chunk(c):
                """Stream-major passes over prefetched x pieces."""
                cs = slice(c * 512, (c + 1) * 512)
                xts = xt_tiles.pop(c)
                kvps = psB.tile([128, 512], f32, tag="kvps")
                q01ps = psB.tile([128, 512], f32, tag="q01ps")
                q23ps = psB.tile([128, 512], f32, tag="q23ps")
                for k in range(16):
                    nc.tensor.matmul(
                        q01ps, lhsT=wq_sb[:, k, 0:128], rhs=xts[k // 2][:, k % 2, :],
                        start=(k == 0), stop=(k == 15),
                    )
                nc.vector.tensor_copy(out=q01[:, cs], in_=q01ps)
                rope_chunk(q01, cs, 128)
                nc.gpsimd.dma_start(out=qx1[:, cs], in_=q01[DH:128, cs])
                for k in range(16):
                    nc.tensor.matmul(
                        kvps, lhsT=wkv_sb[:, k, :], rhs=xts[k // 2][:, k % 2, :],
                        start=(k == 0), stop=(k == 15),
                    )
                nc.vector.tensor_copy(out=kk[:, cs], in_=kvps[0:DH, :])
                nc.vector.tensor_copy(out=vT64[DH:128, cs], in_=kvps[DH:128, :])
                rope_chunk(kk, cs, DH)
                if c < 3:
                    proj_dma(c + 1)
                for st in range(c * 4, c * 4 + 4):
                    vtps = psB.tile([128, DH], f32, tag="kvps")
                    nc.tensor.transpose(
                        vtps, vT64[DH:128, st * 128 : (st + 1) * 128],
                        idv[DH:128, :],
                    )
                    nc.vector.tensor_copy(out=v_sb[:, st, 0:DH], in_=vtps)
                for k in range(16):
                    nc.tensor.matmul(
                        q23ps, lhsT=wq_sb[:, k, 128:256], rhs=xts[k // 2][:, k % 2, :],
                        start=(k == 0), stop=(k == 15),
                    )
                nc.vector.tensor_copy(out=q23[:, cs], in_=q23ps)
                rope_chunk(q23, cs, 128)
                nc.gpsimd.dma_start(out=qx3[:, cs], in_=q23[DH:128, cs])

            # sink + c0-keys, u0-half only: feed ACT right after chunk 0
            EARLY = [(NST - 1, 0), (0, 0), (1, 0), (2, 0), (3, 0)]
            # the matching u1 halves + c1 keys (after chunk 1)
            FRONT2 = [(NST - 1, 1), (0, 1), (1, 1), (2, 1), (3, 1),
                      4, 5, 6, 7]
            BACK = list(range(8, 16))                   # keys from c2/c3
            proj_dma(0)
            # dummy matmuls span the initial DMA wait so the PE p-state ramp
            # is warm when the real stream arrives
            wps = psB.tile([128, 512], f32, tag="q01ps")
            for w in range(18):
                nc.tensor.matmul(wps[0:16, :], lhsT=warm[:, 0:16], rhs=warm,
                                 start=True, stop=True)
            proj_chunk(0)
            _interleave(scores_gen(0, EARLY))
            _interleave(scores_gen(1, EARLY))
            proj_chunk(1)
            _interleave(scores_gen(0, FRONT2))
            _interleave(scores_gen(1, FRONT2))
            proj_chunk(2)
            _interleave(scores_gen(0, [8, 9, 10, 11]), scores_gen(1, [8, 9, 10, 11]))
            proj_chunk(3)

        # ---- phase 2: software-pipelined attention + output projection ----
        with (
            tc.tile_pool(name="psC", bufs=1, space="PSUM") as psC,
            tc.tile_pool(name="psY", bufs=2, space="PSUM") as psY,
            tc.tile_pool(name="pLate", bufs=1) as pLate,
        ):
            wo_sb = pLate.tile([128, 2, DMODEL], bf16, tag="wo_sb")
            nc.gpsimd.dma_start(out=wo_sb, in_=wo.rearrange("(a p) n -> p a n", p=128))

            # R0: finish B0/B1 scores
            _interleave(scores_gen(0, [12, 13, 14, 15]), scores_gen(1, [12, 13, 14, 15]))
            # Rounds: sc(B_{k+2}) rides with ctx(B_k); wT bufs=3 keeps their
            # buffers distinct; norm(B_{k-1}) fully precedes ctx(B_k).
            _interleave(ctx_gen(0), (scores_gen(2, ST_ORDER), 1))
            for k in range(1, 6):
                _interleave(norm_gen(k - 1))
                _interleave(ctx_gen(k), (scores_gen(k + 2, ST_ORDER), 1))
            _interleave(norm_gen(5))
            _interleave(ctx_gen(6), (yout_gen(0, range(4)), 1))
            _interleave(norm_gen(6))
            _interleave(ctx_gen(7), (yout_gen(0, range(4, 8)), 1))
            _interleave(norm_gen(7, act_tail=True),
                        (yout_gen(1, range(8), act_tail=True), 2))

    nc.compile()
    return nc


def _host_inputs(x, kv_cache, Wq, Wk, Wv, Wo, start_pos):
    """Build the 8 per-core input dicts."""
    import ml_dtypes

    f32 = np.float32
    bf16 = ml_dtypes.bfloat16
    xT = np.ascontiguousarray(np.asarray(x, f32)[0].T.astype(bf16))  # (feat, tok)

    inv_freq = (1.0 / (10000.0 ** (np.arange(0, DH, 2, dtype=f32) / DH))).astype(f32)
    pos = np.arange(start_pos, start_pos + T, dtype=f32)
    ang = pos[:, None] * inv_freq[None, :]
    cosT = np.cos(ang).T.astype(f32)  # (32, T)
    sinT = np.sin(ang).T.astype(f32)
    cosb = np.ascontiguousarray(np.concatenate([cosT] * 4, axis=0).astype(bf16))
    sinb = np.ascontiguousarray(
        np.concatenate([-sinT, sinT, -sinT, sinT], axis=0).astype(bf16))

    Wq = np.asarray(Wq, f32)
    Wk = np.asarray(Wk, f32)
    Wv = np.asarray(Wv, f32)
    Wo = np.asarray(Wo, f32)
    kv_cache = np.asarray(kv_cache, f32)

    in_maps = []
    for i in range(NKV):
        sink = kv_cache[0, i, :SINK, :]
        sink_kT = np.ascontiguousarray(sink.T)
        in_maps.append(
            {
                "xT": xT,
                "wq": np.ascontiguousarray(
                    Wq[:, i * GROUP * DH : (i + 1) * GROUP * DH].astype(bf16)
                ),
                "wkv": np.ascontiguousarray(
                    np.concatenate(
                        [Wk[:, i * DH : (i + 1) * DH], Wv[:, i * DH : (i + 1) * DH]],
                        axis=1,
                    ).astype(bf16)
                ),
                "wo": np.ascontiguousarray(Wo[i * GROUP * DH : (i + 1) * GROUP * DH, :]),
                "sink_kT": sink_kT,
                "sink_v": np.ascontiguousarray(sink),
                "cosb": cosb,
                "sinb": sinb,
            }
        )
    return in_maps


def run(inputs, trace=False, trace_kwargs=None):
    """Run the 8-core kernel; returns (y, BassKernelResults)."""
    from concourse.bass_utils import run_bass_kernel_spmd

    if "nc" not in _CACHE:
        _CACHE["nc"] = _build_nc()
    nc = _CACHE["nc"]

    start_pos = int(np.asarray(inputs["start_pos"]))
    in_maps = _host_inputs(
        inputs["x"], inputs["kv_cache"], inputs["Wq"], inputs["Wk"], inputs["Wv"],
        inputs["Wo"], start_pos,
    )
    kwargs = {}
    if trace:
        kwargs["trace"] = True
        if trace_kwargs:
            kwargs["trace_kwargs"] = trace_kwargs
    res = run_bass_kernel_spmd(nc, in_maps, core_ids=list(range(NKV)), **kwargs)

    y = res.results[0]["out"].astype(np.float64)
    for i in range(1, NKV):
        y += res.results[i]["out"].astype(np.float64)
    y = (y + np.asarray(inputs["bo"], np.float64)[None, :]).astype(np.float32)
    return y[None], res


def kernel(**inputs):
    y, _ = run(inputs)
    return y
